# revision 9
# baseline (speedup 1.0000x reference)
"""Pairwise cosine-similarity adjacency (exp(-0.5 * cos_sim)) on 8 trn2 cores.

Input : x [4, 4096, 512] fp32
Output: exp(-0.5 * (xn @ xn.T)) per batch -> [4, 4096, 4096] fp32,
        xn = x / max(||x||_row, 1e-8)

Sharding (symmetry-aware): batch b = core // 2. The 4096x4096 adjacency is
symmetric, so in units of 1024x1024 quarter-blocks Q[i][j] (i,j in 0..3) only
a triangle cover is computed on-device; the host mirrors the rest.

  core even (own rows 0..2047 of batch b) computes
      dtop = rows 0..1023    x cols 0..2047    (Q00, Q01)
      dbot = rows 1024..2047 x cols 1024..2047 (Q11)
      outc = [rows 0..1023    x cols 2048..3071 (Q02);
              rows 1024..2047 x cols 3072..4095 (Q13)]
  core odd runs the same SPMD program fed own = rows 2048..4095 and
      cross = rows [1024..2047, 0..1023], producing Q22/Q23/Q33 and
      Q21, Q30.
  Host mirrors Q01.T, Q02.T, Q13.T, Q21.T, Q30.T into the lower copies.

Device pipeline (per core), optimized for the compute regime:
  - input shipped as bf16 (host cast) -> 4 group DMAs of [128, 8, 512]
  - row norms^2 via DVE tensor_tensor_reduce (x*x, add-reduce)
  - inv = 16/||row|| via DVE-only Newton rsqrt (bit trick + 2 iterations);
    nothing but Exp ever runs on ACT so no activation-table reloads
  - transpose+normalize fused in one PE matmul per (tile, k-chunk):
    xT_chunk = x_tile^T @ diag(inv) accumulated into psum banks, then
    copied (cast) to fp8e4 xnT tiles [128, 2, 4096] (k-pair major)
  - gemms: fp8 DoubleRow matmuls (K=256 per instruction, 2 per
    [128, 512] out chunk), accumulating 256*sim in fp32 psum
  - output: ACT Exp with scale=-0.5/256, bias=ln(255)-0.5 emitting uint8
    (u8 = exp(-sim/2) * 255/sqrt(e)); host dequant is one multiply
"""
import sys

sys.path.insert(0, '/opt/trn_rl_repo')

import numpy as np

B, N, D = 4, 4096, 512
N_CORES = 8
R = N // 2      # 2048 own rows per core
Q = N // 4      # 1024 quarter-block size
NW = 1024       # output tile width (2 psum banks)

U8_BIAS = float(np.log(255.0) - 0.5)
U8_SCALE = float(np.exp(0.5) / 255.0)

_compiled = {}


def _build():
    import concourse.mybir as mybir
    import concourse.tile as tile
    from concourse import bacc
    from concourse.masks import make_identity

    fp32 = mybir.dt.float32
    bf16 = mybir.dt.bfloat16
    fp8 = mybir.dt.float8e4
    u8 = mybir.dt.uint8
    u32 = mybir.dt.uint32
    DR = mybir.MatmulPerfMode.DoubleRow
    Exp = mybir.ActivationFunctionType.Exp
    Alu = mybir.AluOpType

    nc = bacc.Bacc(trn_type="TRN2", target_bir_lowering=False, debug=False,
                   num_devices=N_CORES)
    xown = nc.dram_tensor("xown", [R, D], bf16, kind="ExternalInput")
    xcross = nc.dram_tensor("xcross", [R, D], bf16, kind="ExternalInput")
    dtop = nc.dram_tensor("dtop", [Q, 2 * Q], u8, kind="ExternalOutput")
    dbot = nc.dram_tensor("dbot", [Q, Q], u8, kind="ExternalOutput")
    outc = nc.dram_tensor("outc", [2 * Q, Q], u8, kind="ExternalOutput")

    with tile.TileContext(nc) as tc:
        with tc.tile_pool(name="consts", bufs=1) as consts, \
             tc.tile_pool(name="xn_store", bufs=1) as xn_store, \
             tc.tile_pool(name="p1", bufs=2) as p1, \
             tc.tile_pool(name="p1psum", bufs=2, space="PSUM") as p1psum, \
             tc.tile_pool(name="p2psum", bufs=2, space="PSUM") as p2psum, \
             tc.tile_pool(name="p2out", bufs=4) as p2out:

            ident = consts.tile([128, 128], bf16)
            make_identity(nc, ident[:])
            biasb = consts.tile([128, 1], fp32)
            nc.gpsimd.memset(biasb[:], U8_BIAS)

            # xnT k-pair tiles: [128, 2, 4096] fp8, [:, kk, col] = xn[col, kp*256 + kk*128 + p] * 16
            xnTp = [xn_store.tile([128, 2, N], fp8, name=f"xnTp_{kp}")
                    for kp in range(2)]
            # per-group packed norms^2 and inv*16
            sqh = [xn_store.tile([128, 8], fp32, name=f"sqh_{g}") for g in range(4)]
            invh = [xn_store.tile([128, 8], fp32, name=f"invh_{g}") for g in range(4)]
            nt0 = xn_store.tile([128, 8], fp32, name="nt0")
            nt1 = xn_store.tile([128, 8], fp32, name="nt1")

            # raw row tiles, one group of 8 per buffer
            xtg = [xn_store.tile([128, 8, D], bf16, name=f"xtg_{g}")
                   for g in range(4)]

            srcs = [xown, xcross]

            def load_group(g):
                src = srcs[g // 2].ap()[(g % 2) * 1024:(g % 2) * 1024 + 1024, :]
                nc.sync.dma_start(xtg[g][:, :, :],
                                  src.rearrange("(t p) c -> p t c", p=128))

            def norms_group(g):
                for j in range(8):
                    scr = p1.tile([128, D], bf16, tag="scr")
                    nc.vector.tensor_tensor(scr[:], xtg[g][:, j, :],
                                            xtg[g][:, j, :], Alu.mult)
                    nc.vector.tensor_reduce(sqh[g][:, j:j + 1], scr[:],
                                            mybir.AxisListType.X, Alu.add)
                v, y = sqh[g], invh[g]
                nc.vector.tensor_scalar_max(v[:], v[:], 1e-12)
                # y0 = bitcast(magic - (bitcast(v) >> 1)); the DVE ALU is fp32
                # internally, so the integer subtract must run in float domain
                # (values ~1e9 round to +-64 int ulps, fixed by Newton)
                nc.vector.tensor_scalar(
                    nt0[:].bitcast(u32), v[:].bitcast(u32), 1, None,
                    op0=Alu.logical_shift_right)
                nc.vector.tensor_scalar(
                    y[:].bitcast(u32), nt0[:].bitcast(u32),
                    -1.0, 1597463007.0, op0=Alu.mult, op1=Alu.add)
                # two Newton iterations; second folds the *16 scale
                for c1, c2 in ((-0.5, 1.5), (-8.0, 24.0)):
                    nc.vector.tensor_tensor(nt0[:], y[:], y[:], Alu.mult)
                    nc.vector.tensor_tensor(nt1[:], nt0[:], v[:], Alu.mult)
                    nc.vector.tensor_scalar(nt0[:], nt1[:], c1, c2,
                                            op0=Alu.mult, op1=Alu.add)
                    nc.vector.tensor_tensor(y[:], nt0[:], y[:], Alu.mult)

            def transpose_group(g):
                # 2 subgroups of 4 row tiles; per subgroup one psum bank per k
                for j4 in range(2):
                    # one [128,1024] psum tile (2 banks) per k-pair; cols
                    # [0:512] hold k-even, [512:1024] hold k-odd
                    tps = [p1psum.tile([128, 1024], fp32, tag="tp",
                                       name=f"tp_{kp}")
                           for kp in range(2)]
                    diags = []
                    for jj in range(4):
                        j = j4 * 4 + jj
                        dg = p1.tile([128, 128], bf16, tag="diag", bufs=8)
                        nc.vector.tensor_scalar_mul(dg[:], ident[:],
                                                    invh[g][:, j:j + 1])
                        diags.append(dg)
                    for k in range(4):
                        kp, kk = k // 2, k % 2
                        for jj in range(4):
                            j = j4 * 4 + jj
                            nc.tensor.matmul(
                                tps[kp][:, kk * 512 + jj * 128:
                                        kk * 512 + (jj + 1) * 128],
                                lhsT=xtg[g][:, j, k * 128:(k + 1) * 128],
                                rhs=diags[jj][:],
                                start=(jj == 0), stop=(jj == 3))
                    c0 = (g * 8 + j4 * 4) * 128
                    for kp in range(2):
                        nc.vector.tensor_copy(
                            xnTp[kp][:, 0:2, c0:c0 + 512], tps[kp][:])

            def gemm(m, cb, dst, drow0, dcol0):
                """One [128, NW] output tile: rows m*128.., cols cb*1024.."""
                acc = p2psum.tile([128, NW], fp32, tag="acc")
                for nn in range(NW // 512):
                    c = cb * 1024 + nn * 512
                    for kp in range(2):
                        nc.tensor.matmul(
                            acc[:, nn * 512:(nn + 1) * 512],
                            lhsT=xnTp[kp][:, :, m * 128:(m + 1) * 128],
                            rhs=xnTp[kp][:, :, c:c + 512],
                            start=(kp == 0), stop=(kp == 1),
                            perf_mode=DR)
                ot = p2out.tile([128, NW], u8, tag="ot")
                nc.scalar.activation(ot[:], acc[:], Exp,
                                     bias=biasb[:], scale=-0.5 / 256.0)
                nc.sync.dma_start(
                    dst.ap()[drow0:drow0 + 128, dcol0:dcol0 + NW], ot[:])

            for g in range(4):
                load_group(g)

            for g in range(4):
                norms_group(g)
                transpose_group(g)
                if g == 0:                      # Q00: needs cols 0..1023 only
                    for m in range(8):
                        gemm(m, 0, dtop, m * 128, 0)
                elif g == 1:                    # Q01 + Q11
                    for m in range(8):
                        gemm(m, 1, dtop, m * 128, Q)
                    for m in range(8, 16):
                        gemm(m, 1, dbot, (m - 8) * 128, 0)
                elif g == 2:                    # Q02 (even) / Q21 (odd)
                    for m in range(8):
                        gemm(m, 2, outc, m * 128, 0)
                else:                           # Q13 (even) / Q30 (odd)
                    for m in range(8, 16):
                        gemm(m, 3, outc, m * 128, 0)

    nc.compile()
    return nc


def _to_bf16(a):
    import ml_dtypes
    return np.ascontiguousarray(a).astype(ml_dtypes.bfloat16)


def _in_maps(x):
    maps = []
    for c in range(N_CORES):
        b = c // 2
        xb = x[b]
        if c % 2 == 0:
            maps.append({"xown": _to_bf16(xb[0:R]),
                         "xcross": _to_bf16(xb[R:N])})
        else:
            maps.append({"xown": _to_bf16(xb[R:N]),
                         "xcross": _to_bf16(
                             np.concatenate([xb[Q:2 * Q], xb[0:Q]]))})
    return maps


def _assemble(results, out):
    for c in range(N_CORES):
        b, odd = c // 2, c % 2
        o = out[b]
        r0 = odd * 2 * Q                  # own-row offset: 0 or 2048
        dtop = results[c]["dtop"].astype(np.float32) * U8_SCALE
        dbot = results[c]["dbot"].astype(np.float32) * U8_SCALE
        outc = results[c]["outc"].astype(np.float32) * U8_SCALE
        o[r0:r0 + Q, r0:r0 + 2 * Q] = dtop
        o[r0 + Q:r0 + 2 * Q, r0 + Q:r0 + 2 * Q] = dbot
        o[r0 + Q:r0 + 2 * Q, r0:r0 + Q] = dtop[:, Q:2 * Q].T
        # cross cols: even core -> [2048.., 3072..]; odd -> [1024.., 0..]
        ccol = [2 * Q, 3 * Q] if not odd else [Q, 0]
        for half in range(2):
            blk = outc[half * Q:(half + 1) * Q]
            rr = r0 + half * Q
            cc = ccol[half]
            o[rr:rr + Q, cc:cc + Q] = blk
            o[cc:cc + Q, rr:rr + Q] = blk.T
    return out


def kernel(x: np.ndarray) -> np.ndarray:
    from concourse.bass_utils import run_bass_kernel_spmd

    x = np.asarray(x, dtype=np.float32)
    assert x.shape == (B, N, D)

    if "nc" not in _compiled:
        _compiled["nc"] = _build()
    nc = _compiled["nc"]

    res = run_bass_kernel_spmd(nc, _in_maps(x), list(range(N_CORES)))
    out = np.empty((B, N, N), dtype=np.float32)
    return _assemble([res.results[c] for c in range(N_CORES)], out)
